# revision 10
# baseline (speedup 1.0000x reference)
"""Trainium2 Bass kernel for a dense transformer block (pre-LN, single-head
attention + GELU MLP), data-parallel over the batch dim across 8 NeuronCores.

Per-core problem (batch element): x [S=2048, D=512]
    h  = LN(x; g1, b1)
    q, k, v = h @ wq, h @ wk, h @ wv
    scores = q @ k.T / D ; attn = softmax(scores)
    x = x + (attn @ v) @ wp
    h2 = LN(x; g2, b2)
    out = x + gelu(h2 @ w1) @ w2

On-chip layout is feature-major (xT [D, S]) so every matmul contracts over
the partition dim with no transposes.  All matmuls run in bf16 with fp32
PSUM accumulation; the residual stream stays fp32.  LayerNorm reductions
over the feature dim (= partitions) use ones-vector matmuls on the PE;
per-position stats are broadcast back across partitions with a K=1 matmul.
Softmax over keys (= partitions in scoresT layout) skips max-subtraction
(scores are ~N(0, 1/512·||q||·||k||), far from overflow) and folds the
1/denominator in after the attn@v matmul.
"""

import sys

for _p in ("/opt/trn_rl_repo",):
    if _p not in sys.path:
        sys.path.insert(0, _p)

from contextlib import ExitStack

import ml_dtypes
import numpy as np

import concourse.bass as bass
import concourse.tile as tile
from concourse import bacc, mybir
from concourse._compat import with_exitstack
from concourse.bass_utils import run_bass_kernel_spmd

P = 128
N_CORES = 8
FP32 = mybir.dt.float32
BF16 = mybir.dt.bfloat16
EPS = 1e-5


@with_exitstack
def _block_kernel(ctx: ExitStack, tc: tile.TileContext, t, S, D, H):
    """t: dict of dram APs. S tokens, D model dim, H hidden dim."""
    nc = tc.nc
    DC = D // P          # feature chunks (4)
    HC = H // P          # hidden chunks (16)
    SB = S // P          # token blocks (16)
    CW = 512             # free-dim chunk width (matmul N / psum bank)
    NCH = S // CW        # token chunks (4)

    singles = ctx.enter_context(tc.tile_pool(name="singles", bufs=1))
    big = ctx.enter_context(tc.tile_pool(name="big", bufs=2))
    h2p = ctx.enter_context(tc.tile_pool(name="h2p", bufs=2))
    small = ctx.enter_context(tc.tile_pool(name="small", bufs=3))
    ps1 = ctx.enter_context(tc.tile_pool(name="ps1", bufs=4, space="PSUM"))
    psb = ctx.enter_context(tc.tile_pool(name="psb", bufs=1, space="PSUM"))

    # ---- persistent SBUF tensors ----
    x_sb = singles.tile([P, DC, S], FP32)                 # residual stream (feature-major)
    wq_sb = singles.tile([P, DC, D], BF16)
    wk_sb = singles.tile([P, DC, D], BF16)
    wv_sb = singles.tile([P, DC, D], BF16)
    wp_sb = singles.tile([P, DC, D], BF16)
    w1_sb = singles.tile([P, DC, H], BF16)
    w2_sb = singles.tile([P, HC, D], BF16)
    g1_sb = singles.tile([P, DC], FP32)
    b1_sb = singles.tile([P, DC], FP32)
    g2_sb = singles.tile([P, DC], FP32)
    b2_sb = singles.tile([P, DC], FP32)
    qT = singles.tile([P, DC, S], BF16)
    kT = singles.tile([P, DC, S], BF16)
    v_sb = singles.tile([P, SB, D], BF16)                 # token-major V
    avT = singles.tile([P, DC, S], BF16)

    ones_f = singles.tile([P, 1], FP32)
    ones_b = singles.tile([P, 1], BF16)
    ones1_f = singles.tile([1, P], FP32)
    eps1 = singles.tile([1, 1], FP32)
    nc.vector.memset(ones_f, 1.0)
    nc.vector.memset(ones_b, 1.0)
    nc.vector.memset(ones1_f, 1.0)
    nc.vector.memset(eps1, EPS)

    # ---- load inputs ----
    nc.sync.dma_start(x_sb, t["xT"].rearrange("(c p) s -> p c s", p=P))
    nc.sync.dma_start(wq_sb, t["wq"].rearrange("(c p) m -> p c m", p=P))
    nc.sync.dma_start(wk_sb, t["wk"].rearrange("(c p) m -> p c m", p=P))
    nc.sync.dma_start(wv_sb, t["wv"].rearrange("(c p) m -> p c m", p=P))
    nc.sync.dma_start(wp_sb, t["wp"].rearrange("(c p) m -> p c m", p=P))
    nc.sync.dma_start(w1_sb, t["w1"].rearrange("(c p) m -> p c m", p=P))
    nc.sync.dma_start(w2_sb, t["w2"].rearrange("(c p) m -> p c m", p=P))
    nc.sync.dma_start(g1_sb, t["g1"].rearrange("(c p) -> p c", p=P))
    nc.sync.dma_start(b1_sb, t["b1"].rearrange("(c p) -> p c", p=P))
    nc.sync.dma_start(g2_sb, t["g2"].rearrange("(c p) -> p c", p=P))
    nc.sync.dma_start(b2_sb, t["b2"].rearrange("(c p) -> p c", p=P))

    def ln_chunk(ch, g_sb, b_sb, dst):
        """LayerNorm over features for token chunk ch: dst[:, c, :] (bf16)."""
        sl = slice(ch * CW, (ch + 1) * CW)
        s1 = ps1.tile([1, CW], FP32, name="s1ps", tag="ps")
        s2 = ps1.tile([1, CW], FP32, name="s2ps", tag="ps")
        for c in range(DC):
            xs = x_sb[:, c, sl]
            sq = small.tile([P, CW], BF16, name="sqt", bufs=2)
            nc.scalar.activation(sq, xs, mybir.ActivationFunctionType.Square)
            nc.tensor.matmul(s1, ones_f, xs, start=(c == 0), stop=(c == DC - 1),
                             skip_group_check=True)
            nc.tensor.matmul(s2, ones_b, sq, start=(c == 0), stop=(c == DC - 1),
                             skip_group_check=True)
        a_t = small.tile([1, CW], FP32, name="a_t", bufs=1)
        b_t = small.tile([1, CW], FP32, name="b_t", bufs=1)
        mu = small.tile([1, CW], FP32, name="mut", bufs=1)
        sd = small.tile([1, CW], FP32, name="sdt", bufs=1)
        nc.vector.tensor_scalar_mul(mu, s1, 1.0 / D)              # mu
        nc.vector.tensor_scalar_mul(a_t, s2, 1.0 / D)             # E[x^2]
        nc.vector.tensor_mul(sd, mu, mu)                          # mu^2
        nc.vector.tensor_tensor(a_t, a_t, sd, mybir.AluOpType.subtract)
        nc.scalar.activation(sd, a_t, mybir.ActivationFunctionType.Sqrt,
                             bias=eps1)                           # sqrt(var+eps)
        nc.vector.reciprocal(a_t, sd)                             # A = rstd
        nc.vector.tensor_mul(b_t, mu, a_t)                        # B = mu*rstd
        a_b = ps1.tile([P, CW], FP32, name="abps", tag="ps")
        b_b = ps1.tile([P, CW], FP32, name="bbps", tag="ps")
        nc.tensor.matmul(a_b, ones1_f, a_t, start=True, stop=True)
        nc.tensor.matmul(b_b, ones1_f, b_t, start=True, stop=True)
        for c in range(DC):
            dc = dst[:, c, :]
            nc.vector.tensor_mul(dc, x_sb[:, c, sl], a_b)
            nc.vector.tensor_tensor(dc, dc, b_b, mybir.AluOpType.subtract)
            nc.vector.tensor_scalar(dc, dc,
                                    g_sb[:, c:c + 1], b_sb[:, c:c + 1],
                                    mybir.AluOpType.mult, mybir.AluOpType.add)

    # ============ LN1 + QKV ============
    h1 = big.tile([P, DC, S], BF16, name="h1", tag="big")
    for ch in range(NCH):
        ln_chunk(ch, g1_sb, b1_sb, h1[:, :, ch * CW:(ch + 1) * CW])
    for m in range(DC):
        msl = slice(m * P, (m + 1) * P)
        for ch in range(NCH):
            sl = slice(ch * CW, (ch + 1) * CW)
            qp = ps1.tile([P, CW], FP32, name="qps", tag="ps")
            kp = ps1.tile([P, CW], FP32, name="kps", tag="ps")
            for c in range(DC):
                nc.tensor.matmul(qp, wq_sb[:, c, msl], h1[:, c, sl],
                                 start=(c == 0), stop=(c == DC - 1),
                                 skip_group_check=True)
                nc.tensor.matmul(kp, wk_sb[:, c, msl], h1[:, c, sl],
                                 start=(c == 0), stop=(c == DC - 1),
                                 skip_group_check=True)
            nc.vector.tensor_copy(qT[:, m, sl], qp)
            nc.vector.tensor_copy(kT[:, m, sl], kp)
    for sb_i in range(SB):
        tsl = slice(sb_i * P, (sb_i + 1) * P)
        vp = ps1.tile([P, D], FP32, name="vps", tag="ps")
        for c in range(DC):
            nc.tensor.matmul(vp, h1[:, c, tsl], wv_sb[:, c, :],
                             start=(c == 0), stop=(c == DC - 1))
        nc.vector.tensor_copy(v_sb[:, sb_i, :], vp)

    # ============ attention + proj + residual ============
    for ch in range(NCH):
        sl = slice(ch * CW, (ch + 1) * CW)
        e_t = big.tile([P, SB, CW], BF16, name="e_t", tag="big")
        for skb in range(SB):
            ksl = slice(skb * P, (skb + 1) * P)
            scp = ps1.tile([P, CW], FP32, name="scps", tag="ps")
            for c in range(DC):
                nc.tensor.matmul(scp, kT[:, c, ksl], qT[:, c, sl],
                                 start=(c == 0), stop=(c == DC - 1))
            nc.scalar.activation(e_t[:, skb, :], scp,
                                 mybir.ActivationFunctionType.Exp, scale=1.0 / D)
        dps = ps1.tile([1, CW], FP32, name="dps", tag="ps")
        for skb in range(SB):
            nc.tensor.matmul(dps, ones_b, e_t[:, skb, :],
                             start=(skb == 0), stop=(skb == SB - 1))
        rec = small.tile([1, CW], FP32, name="rec", bufs=1)
        nc.vector.reciprocal(rec, dps)
        rbp = ps1.tile([P, CW], FP32, name="rbp", tag="ps")
        nc.tensor.matmul(rbp, ones1_f, rec, start=True, stop=True)
        rb_sb = small.tile([P, CW], FP32, name="rbs", bufs=2)
        nc.scalar.copy(rb_sb, rbp)
        u = psb.tile([P, DC, CW], FP32, name="ups", tag="psb")
        for m in range(DC):
            msl = slice(m * P, (m + 1) * P)
            for skb in range(SB):
                nc.tensor.matmul(u[:, m, :], v_sb[:, skb, msl], e_t[:, skb, :],
                                 start=(skb == 0), stop=(skb == SB - 1))
        for m in range(DC):
            nc.vector.tensor_mul(avT[:, m, sl], u[:, m, :], rb_sb)
        for m in range(DC):
            msl = slice(m * P, (m + 1) * P)
            pp = ps1.tile([P, CW], FP32, name="pps", tag="ps")
            for c in range(DC):
                nc.tensor.matmul(pp, wp_sb[:, c, msl], avT[:, c, sl],
                                 start=(c == 0), stop=(c == DC - 1))
            nc.vector.tensor_add(x_sb[:, m, sl], x_sb[:, m, sl], pp)

    # ============ LN2 + MLP + residual ============
    for ch in range(NCH):
        sl = slice(ch * CW, (ch + 1) * CW)
        h2 = h2p.tile([P, DC, CW], BF16, name="h2")
        ln_chunk(ch, g2_sb, b2_sb, h2)
        g_t = big.tile([P, HC, CW], BF16, name="g_t", tag="big")
        for hm in range(HC):
            hsl = slice(hm * P, (hm + 1) * P)
            mp = ps1.tile([P, CW], FP32, name="mps", tag="ps")
            for c in range(DC):
                nc.tensor.matmul(mp, w1_sb[:, c, hsl], h2[:, c, :],
                                 start=(c == 0), stop=(c == DC - 1))
            nc.scalar.activation(g_t[:, hm, :], mp,
                                 mybir.ActivationFunctionType.Gelu)
        m2 = psb.tile([P, DC, CW], FP32, name="m2ps", tag="psb")
        for m in range(DC):
            msl = slice(m * P, (m + 1) * P)
            for hm in range(HC):
                nc.tensor.matmul(m2[:, m, :], w2_sb[:, hm, msl], g_t[:, hm, :],
                                 start=(hm == 0), stop=(hm == HC - 1))
        for m in range(DC):
            o_t = small.tile([P, CW], FP32, name="ot", bufs=3)
            nc.vector.tensor_add(o_t, x_sb[:, m, sl], m2[:, m, :])
            nc.sync.dma_start(
                t["outT"].rearrange("(c p) s -> p c s", p=P)[:, m, sl], o_t)


_CACHE = {}


def _build(S, D, H):
    key = (S, D, H)
    if key in _CACHE:
        return _CACHE[key]
    nc = bacc.Bacc("TRN2", target_bir_lowering=False, debug=False,
                   num_devices=N_CORES)
    t = {}
    t["xT"] = nc.dram_tensor("xT", [D, S], FP32, kind="ExternalInput").ap()
    for w, shp in (("wq", [D, D]), ("wk", [D, D]), ("wv", [D, D]),
                   ("wp", [D, D]), ("w1", [D, H]), ("w2", [H, D])):
        t[w] = nc.dram_tensor(w, shp, BF16, kind="ExternalInput").ap()
    for g in ("g1", "b1", "g2", "b2"):
        t[g] = nc.dram_tensor(g, [D], FP32, kind="ExternalInput").ap()
    t["outT"] = nc.dram_tensor("outT", [D, S], FP32, kind="ExternalOutput").ap()

    with tile.TileContext(nc) as tc:
        _block_kernel(tc, t, S, D, H)
    nc.compile()
    _CACHE[key] = nc
    return nc


def _in_maps(x, wq, wk, wv, wp, w1, w2, g1, b1, g2, b2):
    bf = ml_dtypes.bfloat16
    shared = {
        "wq": np.ascontiguousarray(wq.astype(bf)),
        "wk": np.ascontiguousarray(wk.astype(bf)),
        "wv": np.ascontiguousarray(wv.astype(bf)),
        "wp": np.ascontiguousarray(wp.astype(bf)),
        "w1": np.ascontiguousarray(w1.astype(bf)),
        "w2": np.ascontiguousarray(w2.astype(bf)),
        "g1": np.ascontiguousarray(g1, dtype=np.float32),
        "b1": np.ascontiguousarray(b1, dtype=np.float32),
        "g2": np.ascontiguousarray(g2, dtype=np.float32),
        "b2": np.ascontiguousarray(b2, dtype=np.float32),
    }
    maps = []
    for i in range(N_CORES):
        m = dict(shared)
        m["xT"] = np.ascontiguousarray(np.asarray(x[i], dtype=np.float32).T)
        maps.append(m)
    return maps


def run(x, wq, wk, wv, wp, w1, w2, g1, b1, g2, b2, **kwargs):
    """Build + run on 8 cores; returns (output [B,S,D], BassKernelResults)."""
    x = np.asarray(x)
    B, S, D = x.shape
    H = np.asarray(w1).shape[1]
    assert B == N_CORES
    nc = _build(S, D, H)
    maps = _in_maps(x, wq, wk, wv, wp, w1, w2, g1, b1, g2, b2)
    res = run_bass_kernel_spmd(nc, maps, core_ids=list(range(N_CORES)), **kwargs)
    out = np.empty((B, S, D), dtype=np.float32)
    for i in range(N_CORES):
        out[i] = res.results[i]["outT"].T
    return out, res


def kernel(x, wq, wk, wv, wp, w1, w2, g1, b1, g2, b2):
    out, _ = run(x, wq, wk, wv, wp, w1, w2, g1, b1, g2, b2)
    return out
